# revision 14
# baseline (speedup 1.0000x reference)
"""AdditiveAttention Trainium2 kernel (8 NeuronCores, SPMD, no collectives).

reference:
    q = queries @ Wq               (B,Q,H)
    k = keys @ Wk                  (B,K,H)
    scores[b,q,k] = sum_h wv[h] * tanh(q[b,q,h] + k[b,k,h])
    masked = where(arange(K) < valid_lens[b], scores, 0.0)
    attn = softmax(masked, -1)      # masked cols contribute e^0 = 1
    out = attn @ values             (B,Q,D)

Sharding: core c = (b, q_half) -> computes out[b, qh*128:(qh+1)*128, :].
Each core owns 128 queries x full K of one batch. Purely data-parallel,
no cross-core reduction.

Per-core structure (h=H=128 on partitions for the score stage):
  - kpT[h, k], qpT[h, q] via PE matmuls (bf16 in, f32 accum)
  - per q: ONE ScalarE activation computes tanh(kpT + qpT[:, q]) fused
    (per-partition bias), output bf16 [128, KE]
  - per q: PE matmuls with a 32-wide "sliding window" stationary operand
    (wv at column q%32, zeros elsewhere) accumulate that q's scores into
    row q (partition base 32*(q//32)) of a PSUM tile -> dense scores[q,k]
  - per 64-row half: mask multiply (masked logits -> 0), exp on ScalarE
    with accum_out giving the softmax denominator for free, PE transpose
    of E -> E_T; the first half overlaps the second half's tanh stream
  - attn@V matmuls (bf16); k >= KE tail handled by all-ones stationary
    operand (exp(0) == 1 there); normalize with per-partition 1/Z

KE = ceil(max(valid_lens)/128)*128 <= K: columns >= KE are masked in every
batch, so tanh/exp work shrinks to KE columns.
"""

import sys

sys.path.insert(0, "/opt/trn_rl_repo")

from contextlib import ExitStack

import numpy as np
import ml_dtypes

import concourse.bass as bass
import concourse.mybir as mybir
import concourse.tile as tile
from concourse import bacc
from concourse.bass_utils import run_bass_kernel_spmd
from concourse.masks import make_identity

B, Q, K, D, H = 4, 256, 1024, 512, 128
QS = Q // 2  # queries per core
N_CORES = 8
F32 = mybir.dt.float32
BF16 = mybir.dt.bfloat16
BF16_NP = np.dtype(ml_dtypes.bfloat16)
WU_MM = 9  # PE warmup matmuls (~3.8us cold, under the input-DMA shadow)


def build_graph(KE: int) -> bass.Bass:
    assert KE % 128 == 0 and 128 <= KE <= K
    DC = D // 128  # contraction chunks for the projections
    # n-chunks (<=512) of the score/exp free axis
    k_chunks = [(s, min(512, KE - s)) for s in range(0, KE, 512)]
    KC128 = KE // 128
    VC = K // 128
    HQ = QS // 2  # epilogue half

    nc = bacc.Bacc("TRN2", target_bir_lowering=False, debug=False)

    qT_d = nc.declare_dram_parameter("qT", [D, QS], BF16, isOutput=False)
    kT_d = nc.declare_dram_parameter("kT", [D, KE], BF16, isOutput=False)
    v_d = nc.declare_dram_parameter("v", [K, D], BF16, isOutput=False)
    wq_d = nc.declare_dram_parameter("wq", [D, H], BF16, isOutput=False)
    wk_d = nc.declare_dram_parameter("wk", [D, H], BF16, isOutput=False)
    # [128, 128] bf16 sliding windows: col 30 = wv (even q%32), col 64+31 = wv
    wvwin_d = nc.declare_dram_parameter("wvwin", [H, 128], BF16, isOutput=False)
    mask_d = nc.declare_dram_parameter("mask", [QS // 2, KE], BF16, isOutput=False)
    out_d = nc.declare_dram_parameter("out", [QS, D], F32, isOutput=True)

    with tile.TileContext(nc) as tc, ExitStack() as ctx:
        const = ctx.enter_context(tc.tile_pool(name="const", bufs=1))
        work = ctx.enter_context(tc.tile_pool(name="work", bufs=1))
        tq_pool = ctx.enter_context(tc.tile_pool(name="tq", bufs=4))
        pp = ctx.enter_context(tc.tile_pool(name="pp", bufs=1, space="PSUM"))
        scp = ctx.enter_context(tc.tile_pool(name="scp", bufs=1, space="PSUM"))
        tpp = ctx.enter_context(tc.tile_pool(name="tpp", bufs=2, space="PSUM"))
        pop = ctx.enter_context(tc.tile_pool(name="pop", bufs=1, space="PSUM"))

        # ---- load inputs ----
        qT_sb = [const.tile([128, QS], BF16, tag=f"qT{i}", name=f"qT{i}") for i in range(DC)]
        kT_sb = [const.tile([128, KE], BF16, tag=f"kT{i}", name=f"kT{i}") for i in range(DC)]
        wq_sb = [const.tile([128, H], BF16, tag=f"wq{i}", name=f"wq{i}") for i in range(DC)]
        wk_sb = [const.tile([128, H], BF16, tag=f"wk{i}", name=f"wk{i}") for i in range(DC)]
        v_sb = [const.tile([128, D], BF16, tag=f"v{i}", name=f"v{i}") for i in range(VC)]
        wvwin_sb = const.tile([H, 128], BF16, tag="wvwin")
        mask_sb = const.tile([QS // 2, KE], BF16, tag="mask")
        for i in range(VC):
            nc.sync.dma_start(v_sb[i][:], v_d[i * 128 : (i + 1) * 128, :])
        for i in range(DC):
            sl = slice(i * 128, (i + 1) * 128)
            nc.sync.dma_start(kT_sb[i][:], kT_d[sl, :])
            nc.sync.dma_start(wk_sb[i][:], wk_d[sl, :])
            nc.sync.dma_start(qT_sb[i][:], qT_d[sl, :])
            nc.sync.dma_start(wq_sb[i][:], wq_d[sl, :])
        nc.sync.dma_start(wvwin_sb[:], wvwin_d[:, :])
        nc.sync.dma_start(mask_sb[:], mask_d[:, :])

        # ---- PE warmup burst (HAM un-throttle) under the DMA shadow ----
        wu_ps = pp.tile([128, 512], F32, tag="qp_ps", name="wu_ps")
        for i in range(WU_MM):
            nc.tensor.matmul(
                wu_ps[:], v_sb[0][:, :128], v_sb[0][:], start=True, stop=True
            )

        # ---- projections: kpT[h, k] first (gates the tanh stream) ----
        kp_ps = pp.tile([H, KE], F32, tag="kp_ps")
        for s, w in k_chunks:
            for i in range(DC):
                nc.tensor.matmul(
                    kp_ps[:, s : s + w],
                    wk_sb[i][:],
                    kT_sb[i][:, s : s + w],
                    start=(i == 0),
                    stop=(i == DC - 1),
                )
        kp_sb = work.tile([H, KE], F32, tag="kp_sb")
        nc.vector.tensor_copy(kp_sb[:], kp_ps[:])
        qp_ps = pp.tile([H, QS], F32, tag="qp_ps")
        for i in range(DC):
            nc.tensor.matmul(
                qp_ps[:], wq_sb[i][:], qT_sb[i][:], start=(i == 0), stop=(i == DC - 1)
            )
        qp_sb = work.tile([H, QS], F32, tag="qp_sb")
        nc.vector.tensor_copy(qp_sb[:], qp_ps[:])

        ident = const.tile([128, 128], BF16, tag="ident")
        make_identity(nc, ident[:])
        ones_sb = const.tile([128, 128], BF16, tag="ones")
        nc.gpsimd.memset(ones_sb[:], 1.0)

        et_sb = work.tile([128, KC128 * 128], BF16, tag="et_sb")
        z_full = work.tile([QS, 1], F32, tag="z_full")

        def q_block(q, sc_h):
            """tanh + score scatter for one query row (row q % 64 of sc_h)."""
            tq = tq_pool.tile([H, KE], BF16, tag="tq", name="tq")
            nc.scalar.activation(
                tq[:],
                kp_sb[:],
                mybir.ActivationFunctionType.Tanh,
                bias=qp_sb[:, q : q + 1],
            )
            g, r = divmod(q % HQ, 32)
            off = (30 - r) if r % 2 == 0 else (64 + 31 - r)
            win = wvwin_sb[:, off : off + 32]
            for s, w in k_chunks:
                nc.tensor.matmul(
                    sc_h[g * 32 : (g + 1) * 32, s : s + w],
                    win,
                    tq[:, s : s + w],
                    start=(r == 0),
                    stop=(r == 31),
                    tile_position=(0, g * 32),
                )

        def epilogue_half(h, sc_h):
            """mask + exp + transpose for query rows [64h, 64h+64).

            All tiles here live on partitions 0:64 (engines cannot shift
            partitions); the q-offset reappears as a column offset in et_sb
            and via a partition-shifting SBUF->SBUF DMA for z.
            """
            msk_h = work.tile([HQ, KE], F32, tag=f"msk{h}", name=f"msk{h}")
            nc.vector.tensor_mul(msk_h[:], sc_h[:], mask_sb[:])
            e_h = work.tile([HQ, KE], BF16, tag=f"e{h}", name=f"e{h}")
            z_h = work.tile([HQ, 1], F32, tag=f"z{h}", name=f"z{h}")
            nc.scalar.activation(
                e_h[:],
                msk_h[:],
                mybir.ActivationFunctionType.Exp,
                accum_out=z_h[:],
            )
            nc.sync.dma_start(z_full[h * HQ : (h + 1) * HQ, :], z_h[:])
            for c in range(KC128):
                tp = tpp.tile([128, HQ], BF16, tag="tp", name="tp")
                nc.tensor.transpose(
                    tp[:], e_h[:, c * 128 : (c + 1) * 128], ident[:HQ, :HQ]
                )
                nc.vector.tensor_copy(
                    et_sb[:, c * 128 + h * HQ : c * 128 + (h + 1) * HQ], tp[:]
                )

        # per-half PSUM score tiles -> disjoint banks, so the h0 epilogue can
        # read its scores while PE still accumulates h1 (same-bank PE-W +
        # engine-R is a hardware race)
        sc_h0 = scp.tile([HQ, KE], F32, tag="sc_ps")
        for q in range(HQ):
            q_block(q, sc_h0)
        epilogue_half(0, sc_h0)
        sc_h1 = pp.tile([HQ, KE], F32, tag="kp_ps", name="sc_h1")
        for q in range(HQ, QS):
            q_block(q, sc_h1)
        epilogue_half(1, sc_h1)

        # ---- attn @ V  (tail chunks use all-ones: exp(0) = 1) ----
        po = pop.tile([QS, D], F32, tag="po")
        for c in range(VC):
            lhsT = et_sb[:, c * 128 : (c + 1) * 128] if c < KC128 else ones_sb[:]
            nc.tensor.matmul(
                po[:],
                lhsT,
                v_sb[c][:],
                start=(c == 0),
                stop=(c == VC - 1),
            )

        # ---- normalize and store ----
        z2 = work.tile([QS, 1], F32, tag="z2")
        nc.vector.tensor_scalar_add(z2[:], z_full[:], float(K - KE))
        rz = work.tile([QS, 1], F32, tag="rz")
        nc.vector.reciprocal(rz[:], z2[:])
        out_sb = work.tile([QS, D], F32, tag="out_sb")
        nc.vector.tensor_scalar_mul(out_sb[:], po[:], rz[:])
        nc.sync.dma_start(out_d[:, :], out_sb[:])

    nc.compile()
    return nc


_GRAPH_CACHE: dict[int, bass.Bass] = {}
_LAST_RESULTS = None


def _get_graph(KE: int) -> bass.Bass:
    if KE not in _GRAPH_CACHE:
        _GRAPH_CACHE[KE] = build_graph(KE)
    return _GRAPH_CACHE[KE]


def make_in_maps(queries, keys, values, Wq, Wk, wv, valid_lens, KE):
    wvwin = np.zeros((H, 128), BF16_NP)
    wvwin[:, 30] = wv.astype(BF16_NP)
    wvwin[:, 64 + 31] = wv.astype(BF16_NP)
    col = np.arange(KE)
    in_maps = []
    for c in range(N_CORES):
        b, qh = divmod(c, 2)
        mask_row = (col < int(valid_lens[b])).astype(np.float32)
        in_maps.append(
            {
                "qT": np.ascontiguousarray(
                    queries[b, qh * QS : (qh + 1) * QS, :].T.astype(BF16_NP)
                ),
                "kT": np.ascontiguousarray(keys[b, :KE, :].T.astype(BF16_NP)),
                "v": np.ascontiguousarray(values[b].astype(BF16_NP)),
                "wq": np.ascontiguousarray(Wq.astype(BF16_NP)),
                "wk": np.ascontiguousarray(Wk.astype(BF16_NP)),
                "wvwin": wvwin,
                "mask": np.ascontiguousarray(
                    np.broadcast_to(mask_row, (QS // 2, KE)).astype(BF16_NP)
                ),
            }
        )
    return in_maps


def kernel(queries, keys, values, Wq, Wk, wv, valid_lens, **run_kwargs):
    queries = np.asarray(queries, np.float32)
    keys = np.asarray(keys, np.float32)
    values = np.asarray(values, np.float32)
    Wq = np.asarray(Wq, np.float32)
    Wk = np.asarray(Wk, np.float32)
    wv = np.asarray(wv, np.float32)
    valid_lens = np.asarray(valid_lens, np.int32)

    KE = int(-(-int(valid_lens.max()) // 128) * 128)
    KE = max(128, min(K, KE))

    nc = _get_graph(KE)
    in_maps = make_in_maps(queries, keys, values, Wq, Wk, wv, valid_lens, KE)
    res = run_bass_kernel_spmd(
        nc, in_maps, core_ids=list(range(N_CORES)), **run_kwargs
    )
    global _LAST_RESULTS
    _LAST_RESULTS = res
    out = np.empty((B, Q, D), np.float32)
    for c in range(N_CORES):
        b, qh = divmod(c, 2)
        out[b, qh * QS : (qh + 1) * QS, :] = res.results[c]["out"]
    return out
